# revision 5
# baseline (speedup 1.0000x reference)
"""Causal MHA (B=4, S=2048, D=1024, H=16) on 8 TRN2 cores — v2.

Core = (batch, head-group of 8 heads); each core computes its batch's
attention for its heads plus the partial output projection; the host
sums the two per-batch partials. ~600us/core (baseline 1032us).

Key ideas vs the v1 baseline:
  - Matmul cost on TRN2 is output-width based and independent of the
    contraction depth, so the unused 64 PE partitions carry the second
    hi/lo split term: rope'd Q/K live in SBUF as packed bf16 tiles
      Kpack   = [Khi; Klo]            QhiQhi = [Qhi; Qhi]
      KhiAug  = [Khi; 1; 1] (66 rows) QloAug = [Qlo; -mh; -ml]
    and pass-2 scores take 2 matmuls instead of 3:
      m1: Kpack x QhiQhi -> Khi.Qhi + Klo.Qhi
      m2: KhiAug x QloAug -> Khi.Qlo + shift
    (the softmax shift is a bf16 hi+lo pair since scores reach ~22k;
    pass-1 row max = QhiQhi x Kpack = Qhi.K, one matmul per tile).
  - No DRAM roundtrip: rope writes hi terms straight into pack tiles
    (Q) or a staging tile (K, full-width DVE ops); lo/dup/aug rows
    move via SBUF-SBUF DMAs on the idle SP HWDGE queue (~.6us) rather
    than Pool SWDGE (~1us of Pool engine each).
  - Diagonal kv-tiles only compute columns >= their triangle quarter
    (the rest is fully masked); masks are 128-wide quarter matmuls.
  - Emission is a software pipeline: pass-1 of (h,J+1), deferred
    normalization of (h,J-1) (rank-1 1/den broadcast via matmul), then
    pass-2+AV of (h,J); projections+rope of the next head pair are
    interleaved at fixed points so pack rings stay at 3 buffers while
    the PE never starves; per-chunk x loads feed the first projection
    as they arrive.
"""

import numpy as np

P = 128
B, S_FULL, DM = 4, 2048, 1024
H, DK = 16, 64
NG = 2
HPG = 8
DG = 512
THETA = 10000.0
MASK_VAL = -1e9


def build_nc(S):
    import concourse.bacc as bacc
    import concourse.mybir as mybir
    import concourse.tile as tile

    dt = mybir.dt
    ST = S // P
    NJ = ST // 4
    SC = S // 512

    TC = tile.TileContext
    nc = bacc.Bacc(None, target_bir_lowering=False)

    xTh = nc.dram_tensor("xTh", [DM, S], dt.bfloat16, kind="ExternalInput")
    xTl = nc.dram_tensor("xTl", [DM, S], dt.bfloat16, kind="ExternalInput")
    qkwh = nc.dram_tensor("qkwh", [DM, DM], dt.bfloat16, kind="ExternalInput")
    qkwl = nc.dram_tensor("qkwl", [DM, DM], dt.bfloat16, kind="ExternalInput")
    vw = nc.dram_tensor("vw", [DM, DG], dt.bfloat16, kind="ExternalInput")
    ow = nc.dram_tensor("ow", [DG, DM], dt.bfloat16, kind="ExternalInput")
    cs = nc.dram_tensor("cs", [P, 2, S], dt.float32, kind="ExternalInput")
    idr = nc.dram_tensor("idr", [P, P], dt.bfloat16, kind="ExternalInput")
    idrf = nc.dram_tensor("idrf", [P, P], dt.float32r, kind="ExternalInput")
    mkm = nc.dram_tensor("mkm", [P, P], dt.bfloat16, kind="ExternalInput")
    md = nc.dram_tensor("md", [P, 4, P], dt.bfloat16, kind="ExternalInput")
    on1 = nc.dram_tensor("on1", [P, DK], dt.float32r, kind="ExternalInput")
    opart = nc.dram_tensor("opart", [S, DM], dt.float32, kind="ExternalOutput")

    AluOp = mybir.AluOpType
    Act = mybir.ActivationFunctionType
    AxX = mybir.AxisListType.X

    import os
    _dma_eng = os.environ.get("K2_DMA_ENGINE", "sync")

    def sdma(out, in_):
        getattr(nc, _dma_eng).dma_start(out, in_)

    with TC(nc) as tc:
        with (
            tc.tile_pool(name="cp", bufs=1) as cp,
            tc.tile_pool(name="ps", bufs=1, space="PSUM") as ps,
        ):
            idrsb = cp.tile([P, P], dt.bfloat16, tag="idr", name="idrsb")
            idrr = cp.tile([P, P], dt.float32r, tag="idrr", name="idrr")
            mkmsb = cp.tile([P, P], dt.bfloat16, tag="mkm", name="mkmsb")
            mdsb = cp.tile([P, 4, P], dt.bfloat16, tag="md", name="mdsb")
            on1sb = cp.tile([P, DK], dt.float32r, tag="on1", name="on1sb")
            cssb = cp.tile([P, 2, S], dt.float32, tag="cssb", name="cssb")
            vaug = cp.tile([P, ST, HPG, DK + 1], dt.bfloat16, tag="vaug",
                           name="vaug")
            nc.gpsimd.memset(vaug[:, :, :, DK : DK + 1], 1.0)

            def load_consts(xp):
                # after the x chunks: cos/sin on the lightly used sync queue,
                # small constants behind the x chunks on gpsimd
                for q4 in range(4):
                    nc.sync.dma_start(cssb[:, :, q4 * 512 : (q4 + 1) * 512],
                                      cs[:, :, q4 * 512 : (q4 + 1) * 512])

                nc.gpsimd.dma_start(idrsb[:], idr[:])
                nc.gpsimd.dma_start(idrr[:], idrf[:])
                nc.gpsimd.dma_start(mkmsb[:], mkm[:])
                nc.gpsimd.dma_start(mdsb[:], md[:])
                nc.gpsimd.dma_start(on1sb[:], on1[:])
            aoT = []
            for pr in range(4):
                t_ = cp.tile([P, S], dt.bfloat16, tag=f"aoT{pr}", name=f"aoT{pr}")
                aoT.append(t_)

            packs = {}      # h -> (Kpack, KhiAug, QhiQhi, QloAug)
            pending_norm = []
            at_pool = [None]

            def emit_norm():
                while pending_norm:
                    h, J, avp, denr, dst = pending_norm.pop(0)
                    rk = ps.tile([DK, 512], dt.float32, tag="rk", bufs=1,
                                 name=f"rk{h}{J}")
                    nc.tensor.matmul(rk[:], lhsT=on1sb[DK : DK + 1, :],
                                     rhs=denr[DK : DK + 1, :],
                                     start=True, stop=True)
                    bc = at_pool[0].tile([DK, 512], dt.float32, tag="bc",
                                         bufs=1, name=f"bc{h}{J}")
                    nc.scalar.copy(bc[:], rk[:])
                    nc.vector.tensor_tensor(dst, avp[0:DK, :], bc[:],
                                            AluOp.mult)
                    if h % 2 == 1:
                        sdma(aoT[h // 2][DK:P, J * 512 : (J + 1) * 512],
                             aotmps[(h, J)][:])

            # ---------- phase A pieces ----------
            def load_inputs(xp):
                xsb, xsl = [], []
                for k in range(8):
                    eng = nc.gpsimd if k % 2 == 0 else nc.scalar
                    th = xp.tile([P, S], dt.bfloat16, tag=f"xsb{k}",
                                 name=f"xsb{k}")
                    eng.dma_start(th[:], xTh[k * P : (k + 1) * P, :])
                    xsb.append(th)
                    tl_ = xp.tile([P, S], dt.bfloat16, tag=f"xsl{k}",
                                  name=f"xsl{k}")
                    eng.dma_start(tl_[:], xTl[k * P : (k + 1) * P, :])
                    xsl.append(tl_)
                return xsb, xsl

            def v_proj(xp, xsb):
                with tc.tile_pool(name="vp", bufs=1) as vp:
                    vwsb = vp.tile([P, 8, DG], dt.bfloat16, tag="vwsb",
                                   name="vwsb")
                    nc.gpsimd.dma_start(
                        vwsb[:], vw.rearrange("(kt p) o -> p kt o", p=P))
                    for st in range(ST):
                        vps = ps.tile([P, DG], dt.float32, tag="pj", bufs=2,
                                      name=f"vps{st}")
                        for k in range(8):
                            nc.tensor.matmul(
                                vps[:],
                                lhsT=xsb[k][:, st * P : (st + 1) * P],
                                rhs=vwsb[:, k, :],
                                start=(k == 0), stop=(k == 7))
                        nc.scalar.copy(
                            vaug[:, st, :, 0:DK],
                            vps[:].rearrange("p (h d) -> p h d", d=DK))

            def proj_rope(xp, xsb, xsl, t, pr, after_weights=None):
                """Project 128 dims (t: 0=q 1=k; pr: head pair), rope, split
                hi/lo into pack tiles for heads hA=2pr, hB=2pr+1."""
                hA, hB = 2 * pr, 2 * pr + 1
                # load weight slices for this (t, pr)
                wh = xp.tile([P, 8, P], dt.bfloat16, tag="wh", bufs=2,
                             name=f"wh{t}{pr}")
                nc.sync.dma_start(
                    wh[:],
                    qkwh[:, t * DG + pr * P : t * DG + (pr + 1) * P].rearrange(
                        "(kt p) o -> p kt o", p=P))
                wl = xp.tile([P, 8, P], dt.bfloat16, tag="wl", bufs=2,
                             name=f"wl{t}{pr}")
                nc.sync.dma_start(
                    wl[:],
                    qkwl[:, t * DG + pr * P : t * DG + (pr + 1) * P].rearrange(
                        "(kt p) o -> p kt o", p=P))
                if after_weights is not None:
                    after_weights()

                # allocate pack tiles for this pair on first (t) visit
                if hA not in packs:
                    for h in (hA, hB):
                        kp = cp.tile([P, S], dt.bfloat16, tag="kpack", bufs=3,
                                     name=f"kpack{h}")
                        ka = cp.tile([DK + 2, S], dt.bfloat16, tag="khiaug",
                                     bufs=3, name=f"khiaug{h}")
                        nc.gpsimd.memset(ka[DK : DK + 2, :], 1.0)
                        qq_ = cp.tile([P, S], dt.bfloat16, tag="qhq", bufs=3,
                                      name=f"qhq{h}")
                        ql = cp.tile([DK + 2, S], dt.bfloat16, tag="qloaug",
                                     bufs=3, name=f"qloaug{h}")
                        packs[h] = (kp, ka, qq_, ql)

                stagl = xp.tile([P, S], dt.bfloat16, tag="staglo", bufs=1,
                                name=f"staglo{t}{pr}")
                stagh = (xp.tile([P, S], dt.bfloat16, tag="stagh", bufs=1,
                                 name=f"stagh{pr}") if t == 1 else None)
                kpA, kaA, qqA, qlA = packs[hA]
                kpB, kaB, qqB, qlB = packs[hB]
                terms = [(wh, xsb), (wh, xsl), (wl, xsb)]
                for cp2 in range(2):
                    pss = [ps.tile([P, 512], dt.float32, tag="pj", bufs=2,
                                   name=f"qps{t}{pr}{cp2}{i}") for i in range(2)]
                    for k in range(8):
                        for ti, (wt, xt) in enumerate(terms):
                            for i in range(2):
                                ch = 2 * cp2 + i
                                nc.tensor.matmul(
                                    pss[i][:],
                                    lhsT=wt[:, k, :],
                                    rhs=xt[k][:, ch * 512 : (ch + 1) * 512],
                                    start=(ti == 0 and k == 0),
                                    stop=(ti == 2 and k == 7))
                    for i in range(2):
                        ch = 2 * cp2 + i
                        sl = slice(ch * 512, (ch + 1) * 512)
                        qraw = xp.tile([P, 512], dt.float32, tag="qraw",
                                       bufs=2, name=f"qraw{t}{pr}{ch}")
                        nc.scalar.copy(qraw[:], pss[i][:])
                        qsw = xp.tile([P, 512], dt.float32, tag="qsw", bufs=2,
                                      name=f"qsw{t}{pr}{ch}")
                        # rotate-half swap (partition +-32 in each 64 block)
                        sdma(qsw[0:32, :], qraw[32:64, :])
                        sdma(qsw[32:64, :], qraw[0:32, :])
                        sdma(qsw[64:96, :], qraw[96:128, :])
                        sdma(qsw[96:128, :], qraw[64:96, :])
                        nc.gpsimd.tensor_tensor(qraw[:], qraw[:],
                                                cssb[:, 0, sl], AluOp.mult)
                        nc.gpsimd.tensor_tensor(qsw[:], qsw[:],
                                                cssb[:, 1, sl], AluOp.mult)
                        if t == 0:
                            # hi = bf16(rot) straight into the qq pack halves
                            nc.vector.tensor_tensor(qqA[0:64, sl],
                                                    qraw[0:64, :],
                                                    qsw[0:64, :], AluOp.add)
                            nc.vector.tensor_tensor(qqB[64:128, sl],
                                                    qraw[64:128, :],
                                                    qsw[64:128, :], AluOp.add)
                            # lo = (rot - hi): qraw -= hi, then + qsw
                            nc.vector.tensor_tensor(qraw[0:64, :],
                                                    qraw[0:64, :],
                                                    qqA[0:64, sl],
                                                    AluOp.subtract)
                            nc.vector.tensor_tensor(qraw[64:128, :],
                                                    qraw[64:128, :],
                                                    qqB[64:128, sl],
                                                    AluOp.subtract)
                            # loA direct into QloAug rows 0:64; loB staged
                            nc.vector.tensor_tensor(qlA[0:64, sl],
                                                    qraw[0:64, :],
                                                    qsw[0:64, :], AluOp.add)
                            nc.vector.tensor_tensor(stagl[64:128, sl],
                                                    qraw[64:128, :],
                                                    qsw[64:128, :], AluOp.add)
                        else:
                            # K side: full-width ops via hi staging (DVE cost
                            # is width-based, so 3 ops instead of 6)
                            nc.vector.tensor_tensor(stagh[:, sl], qraw[:],
                                                    qsw[:], AluOp.add)
                            nc.vector.tensor_tensor(qraw[:], qraw[:],
                                                    stagh[:, sl],
                                                    AluOp.subtract)
                            nc.vector.tensor_tensor(stagl[:, sl], qraw[:],
                                                    qsw[:], AluOp.add)
                # relayout DMAs
                if t == 0:
                    sdma(qqA[64:128, :], qqA[0:64, :])
                    sdma(qqB[0:64, :], qqB[64:128, :])
                    sdma(qlB[0:64, :], stagl[64:128, :])
                else:
                    sdma(kpA[0:64, :], stagh[0:64, :])
                    sdma(kpB[64:128, :], stagh[64:128, :])
                    sdma(kaA[0:DK, :], stagh[0:64, :])
                    sdma(kaB[0:DK, :], stagh[64:128, :])
                    sdma(kpA[64:128, :], stagl[0:64, :])
                    sdma(kpB[0:64, :], stagl[64:128, :])

            # ---------- phase B: attention for one head ----------
            aotmps = {}

            def pass1(at, h, J):
                """Row max of causal scores for q-block J -> QloAug aug rows."""
                kp, ka, qq_, ql = packs[h]
                Jsl = slice(J * 512, (J + 1) * 512)

                negm4 = at.tile([P, 4], dt.float32r, tag="negm4", bufs=2,
                                name=f"negm4{h}{J}")
                for qq in range(4):
                    qi = 4 * J + qq
                    kv = (qi + 1) * P
                    nch = (kv + 511) // 512
                    mparts = []
                    for c in range(nch):
                        cw = min(512, kv - c * 512)
                        sc_ = ps.tile([P, 512], dt.float32, tag="sc",
                                      bufs=2, name=f"sc{h}{qi}{c}")
                        last = c == nch - 1
                        nc.tensor.matmul(
                            sc_[:, 0:cw],
                            lhsT=qq_[:, qi * P : (qi + 1) * P],
                            rhs=kp[:, c * 512 : c * 512 + cw],
                            start=True, stop=not last)
                        if last:
                            doff = qi * P - c * 512
                            nc.tensor.matmul(
                                sc_[:, doff : doff + P],
                                lhsT=idrsb[:], rhs=mkmsb[:],
                                start=False, stop=True)
                        if nch == 1:
                            # single chunk: reduce straight into negm4 column
                            nc.vector.reduce_max(negm4[:, qq : qq + 1],
                                                 sc_[:, 0:cw], axis=AxX,
                                                 negate=True)
                        else:
                            mp = at.tile([P, 1], dt.float32r, tag="mp",
                                         bufs=6, name=f"mp{h}{qi}{c}")
                            nc.vector.reduce_max(mp[:], sc_[:, 0:cw],
                                                 axis=AxX, negate=True)
                            mparts.append(mp)
                    if nch > 1:
                        # fold the partial minima; last op lands in negm4
                        acc = mparts[0]
                        for m2_ in mparts[1:-1]:
                            nc.vector.tensor_tensor(acc[:], acc[:], m2_[:],
                                                    AluOp.min)
                        nc.vector.tensor_tensor(negm4[:, qq : qq + 1],
                                                acc[:], mparts[-1][:],
                                                AluOp.min)
                def emit_shift():
                    # transpose -max to [4,128], split to bf16 hi/lo, one
                    # linearizing DMA into each aug row of QloAug
                    ngt = ps.tile([4, P], dt.float32r, tag="ngt", bufs=1,
                                  name=f"ngt{h}{J}")
                    nc.tensor.transpose(ngt[:], negm4[:], idrr[:])
                    ngh = at.tile([4, P], dt.bfloat16, tag="ngh", bufs=2,
                                  name=f"ngh{h}{J}")
                    ngl = at.tile([4, P], dt.bfloat16, tag="ngl", bufs=2,
                                  name=f"ngl{h}{J}")
                    nc.vector.tensor_copy(ngh[:], ngt[:])
                    nc.vector.tensor_tensor(ngl[:], ngt[:], ngh[:],
                                            AluOp.subtract)
                    sdma(ql[DK : DK + 1, Jsl], ngh[:])
                    sdma(ql[DK + 1 : DK + 2, Jsl], ngl[:])
                return emit_shift

            def pass2(at, h, J, mid_cb=None):
                """Scores^T + exp + AV + denominator for q-block J."""
                kp, ka, qq_, ql = packs[h]
                pr = h // 2
                Jsl = slice(J * 512, (J + 1) * 512)
                avp = ps.tile([DK + 1, 512], dt.float32, tag="avp",
                              bufs=2, name=f"avp{h}{J}")
                nj = 4 * J + 4
                prev = []
                for j in range(nj):
                    dj = j - 4 * J
                    # columns left of a diagonal tile's triangle quarter are
                    # fully masked -- skip computing them
                    c0 = max(dj, 0) * P
                    cw = 512 - c0
                    qsl = slice(J * 512 + c0, (J + 1) * 512)
                    stp = ps.tile([P, 512], dt.float32, tag="pj", bufs=2,
                                  name=f"stp{h}{J}{j}")
                    nc.tensor.matmul(
                        stp[:, c0:512],
                        lhsT=kp[:, j * P : (j + 1) * P],
                        rhs=qq_[:, qsl],
                        start=True, stop=False)
                    nc.tensor.matmul(
                        stp[:, c0:512],
                        lhsT=ka[0 : DK + 2, j * P : (j + 1) * P],
                        rhs=ql[0 : DK + 2, qsl],
                        start=False, stop=(dj < 0),
                        skip_group_check=(dj >= 0))
                    if dj >= 0:
                        nc.tensor.matmul(
                            stp[:, c0 : c0 + P], lhsT=idrsb[:],
                            rhs=mdsb[:, dj, :],
                            start=False, stop=True, skip_group_check=True)
                    att = at.tile([P, 512], dt.bfloat16, tag="att",
                                  bufs=4, name=f"att{h}{J}{j}")
                    nc.scalar.activation(att[:, c0:512], stp[:, c0:512],
                                         Act.Exp)
                    prev.append((j, max(dj, 0) * P, att))
                    if len(prev) > 1:
                        pj_, pc0, patt = prev.pop(0)
                        nc.tensor.matmul(
                            avp[:, pc0:512], lhsT=vaug[:, pj_, h, :],
                            rhs=patt[:, pc0:512],
                            start=(pj_ == 0), stop=False,
                            skip_group_check=True)
                    if j == 1 and mid_cb is not None:
                        mid_cb()
                pj_, pc0, patt = prev.pop(0)
                nc.tensor.matmul(
                    avp[:, pc0:512], lhsT=vaug[:, pj_, h, :],
                    rhs=patt[:, pc0:512],
                    start=(pj_ == 0), stop=True, skip_group_check=True)
                denr = at.tile([DK + 1, 512], dt.float32r, tag="denr",
                               bufs=1, name=f"denr{h}{J}")
                nc.scalar.copy(denr[DK : DK + 1, :], avp[DK : DK + 1, :])
                with nc.allow_low_precision(reason="f32r recip of denom"):
                    nc.vector.reciprocal(denr[DK : DK + 1, :],
                                         denr[DK : DK + 1, :])
                if h % 2 == 0:
                    dst = aoT[pr][0:DK, Jsl]
                else:
                    dst = at.tile([DK, 512], dt.bfloat16, tag="aotmp",
                                  bufs=2, name=f"aotmp{h}{J}")
                    aotmps[(h, J)] = dst
                    dst = dst[:]
                pending_norm.append((h, J, avp, denr, dst))

            # ---------- emission ----------
            with tc.tile_pool(name="xp", bufs=1) as xp:
                with tc.tile_pool(name="at", bufs=1) as at:
                    at_pool[0] = at
                    xsb, xsl = load_inputs(xp)
                    proj_rope(xp, xsb, xsl, 1, 0,
                              after_weights=lambda: load_consts(xp))
                    proj_rope(xp, xsb, xsl, 0, 0)
                    v_proj(xp, xsb)
                    seq = [(h, J) for h in (0, 1, 2, 3, 4, 5, 7, 6)
                           for J in range(NJ)]
                    pass1(at, 0, 0)()
                    for i, (h, J) in enumerate(seq):
                        if i + 1 < len(seq):
                            pass1(at, *seq[i + 1])()
                        emit_norm()
                        pass2(at, h, J)
                        if (h, J) == (0, NJ - 1):
                            proj_rope(xp, xsb, xsl, 0, 1)
                        elif (h, J) == (1, 0):
                            proj_rope(xp, xsb, xsl, 1, 1)
                        elif (h, J) == (2, NJ - 1):
                            proj_rope(xp, xsb, xsl, 0, 2)
                        elif (h, J) == (3, 0):
                            proj_rope(xp, xsb, xsl, 1, 2)
                        elif (h, J) == (4, NJ - 1):
                            proj_rope(xp, xsb, xsl, 0, 3)
                        elif (h, J) == (5, 0):
                            proj_rope(xp, xsb, xsl, 1, 3)
                    emit_norm()

            # ---------- output projection (own pool: reuses freed space) ----
            with tc.tile_pool(name="op", bufs=1) as opp:
                owsb = []
                for pr4 in range(4):
                    t_ = opp.tile([P, DM], dt.bfloat16, tag=f"ow{pr4}",
                                  name=f"owsb{pr4}")
                    eng = nc.gpsimd if pr4 % 2 == 0 else nc.sync
                    eng.dma_start(t_[:], ow[pr4 * P : (pr4 + 1) * P, :])
                    owsb.append(t_)
                for st in range(ST):
                    ops = [ps.tile([P, 512], dt.float32, tag=tg, bufs=2,
                                   name=f"op{st}{tg}") for tg in ("pj", "sc")]
                    for pr4 in range(4):
                        for oc in range(2):
                            nc.tensor.matmul(
                                ops[oc][:],
                                lhsT=aoT[pr4][:, st * P : (st + 1) * P],
                                rhs=owsb[pr4][:, oc * 512 : (oc + 1) * 512],
                                start=(pr4 == 0), stop=(pr4 == 3))
                    osb = opp.tile([P, DM], dt.float32, tag="osb", bufs=3,
                                   name=f"osb{st}")
                    for oc in range(2):
                        nc.scalar.copy(osb[:, oc * 512 : (oc + 1) * 512],
                                       ops[oc][:])
                    sdma(opart[st * P : (st + 1) * P, :],
                                      osb[:])

    nc.compile()
    return nc


def _host_prep(x, q_w, k_w, v_w, o_w, S):
    import ml_dtypes

    perm = np.zeros(DM, dtype=np.int64)
    for h in range(H):
        for i in range(32):
            perm[h * DK + i] = h * DK + 2 * i
            perm[h * DK + 32 + i] = h * DK + 2 * i + 1
    q_wp = (q_w[perm] * 0.125).astype(np.float32)
    k_wp = k_w[perm].astype(np.float32)

    inv_freq = 1.0 / THETA ** (2.0 * np.arange(32, dtype=np.float64) / DK)
    pos = np.arange(S, dtype=np.float64)
    ang = inv_freq[:, None] * pos[None, :]
    cos = np.cos(ang).astype(np.float32)
    sin = np.sin(ang).astype(np.float32)
    cs = np.zeros((P, 2, S), dtype=np.float32)
    for blk in range(2):
        b0 = blk * DK
        cs[b0 : b0 + 32, 0] = cos
        cs[b0 + 32 : b0 + 64, 0] = cos
        cs[b0 : b0 + 32, 1] = -sin
        cs[b0 + 32 : b0 + 64, 1] = sin

    bf = ml_dtypes.bfloat16
    idr = np.eye(P, dtype=np.float32).astype(bf)
    idrf = np.eye(P, dtype=np.float32)
    r = np.arange(P)
    mkm = np.where(r[None, :] > r[:, None], np.float32(MASK_VAL),
                   np.float32(0.0)).astype(bf)
    # only each diagonal tile's own 128-wide triangle quarter is read
    md = np.zeros((P, 4, P), dtype=np.float32)
    tri = np.where(r[None, :] < r[:, None], np.float32(MASK_VAL),
                   np.float32(0.0))
    for dj in range(4):
        md[:, dj, :] = tri
    md = md.astype(bf)
    on1 = np.ones((P, DK), dtype=np.float32)

    in_maps = []
    for b in range(B):
        for g in range(NG):
            rows = slice(g * DG, (g + 1) * DG)
            xt = np.ascontiguousarray(x[b].T)
            xth = xt.astype(bf)
            qkwf = np.ascontiguousarray(
                np.concatenate([q_wp[rows].T, k_wp[rows].T], axis=1))
            qkwhh = qkwf.astype(bf)
            in_maps.append({
                "xTh": xth,
                "xTl": (xt - xth.astype(np.float32)).astype(bf),
                "qkwh": qkwhh,
                "qkwl": (qkwf - qkwhh.astype(np.float32)).astype(bf),
                "vw": np.ascontiguousarray(v_w[rows].T).astype(bf),
                "ow": np.ascontiguousarray(o_w[:, rows].T).astype(bf),
                "cs": cs,
                "idr": idr,
                "idrf": idrf,
                "mkm": mkm,
                "md": md,
                "on1": on1,
            })
    return in_maps


_NC_CACHE = {}


def kernel(x, q_w, k_w, v_w, o_w):
    import sys

    for p in ("/opt/trn_rl_repo",):
        if p not in sys.path:
            sys.path.insert(0, p)
    from concourse.bass_utils import run_bass_kernel_spmd

    x = np.asarray(x, dtype=np.float32)
    q_w = np.asarray(q_w, dtype=np.float32)
    k_w = np.asarray(k_w, dtype=np.float32)
    v_w = np.asarray(v_w, dtype=np.float32)
    o_w = np.asarray(o_w, dtype=np.float32)
    S = x.shape[1]

    if S not in _NC_CACHE:
        _NC_CACHE[S] = build_nc(S)
    nc = _NC_CACHE[S]

    in_maps = _host_prep(x, q_w, k_w, v_w, o_w, S)
    res = run_bass_kernel_spmd(nc, in_maps, core_ids=list(range(8)))

    out = np.zeros((B, S, DM), dtype=np.float32)
    for b in range(B):
        for g in range(NG):
            out[b] += res.results[b * NG + g]["opart"]
    return out


# revision 6
# speedup vs baseline: 1.0004x; 1.0004x over previous
"""Causal MHA (B=4, S=2048, D=1024, H=16) on 8 TRN2 cores — v2.

Core = (batch, head-group of 8 heads); each core computes its batch's
attention for its heads plus the partial output projection; the host
sums the two per-batch partials. ~600us/core (baseline 1032us).

Key ideas vs the v1 baseline:
  - Matmul cost on TRN2 is output-width based and independent of the
    contraction depth, so the unused 64 PE partitions carry the second
    hi/lo split term: rope'd Q/K live in SBUF as packed bf16 tiles
      Kpack   = [Khi; Klo]            QhiQhi = [Qhi; Qhi]
      KhiAug  = [Khi; 1; 1] (66 rows) QloAug = [Qlo; -mh; -ml]
    and pass-2 scores take 2 matmuls instead of 3:
      m1: Kpack x QhiQhi -> Khi.Qhi + Klo.Qhi
      m2: KhiAug x QloAug -> Khi.Qlo + shift
    (the softmax shift is a bf16 hi+lo pair since scores reach ~22k;
    pass-1 row max = QhiQhi x Kpack = Qhi.K, one matmul per tile).
  - No DRAM roundtrip: rope writes hi terms straight into pack tiles
    (Q) or a staging tile (K, full-width DVE ops); lo/dup/aug rows
    move via SBUF-SBUF DMAs on the idle SP HWDGE queue (~.6us) rather
    than Pool SWDGE (~1us of Pool engine each).
  - Diagonal kv-tiles only compute columns >= their triangle quarter
    (the rest is fully masked); masks are 128-wide quarter matmuls.
  - Emission is a software pipeline: pass-1 of (h,J+1), deferred
    normalization of (h,J-1) (rank-1 1/den broadcast via matmul), then
    pass-2+AV of (h,J); projections+rope of the next head pair are
    interleaved at fixed points so pack rings stay at 3 buffers while
    the PE never starves; per-chunk x loads feed the first projection
    as they arrive.
"""

import numpy as np

P = 128
B, S_FULL, DM = 4, 2048, 1024
H, DK = 16, 64
NG = 2
HPG = 8
DG = 512
THETA = 10000.0
MASK_VAL = -1e9


def build_nc(S):
    import concourse.bacc as bacc
    import concourse.mybir as mybir
    import concourse.tile as tile

    dt = mybir.dt
    ST = S // P
    NJ = ST // 4
    SC = S // 512

    TC = tile.TileContext
    nc = bacc.Bacc(None, target_bir_lowering=False)

    xTh = nc.dram_tensor("xTh", [DM, S], dt.bfloat16, kind="ExternalInput")
    xTl = nc.dram_tensor("xTl", [DM, S], dt.bfloat16, kind="ExternalInput")
    qkwh = nc.dram_tensor("qkwh", [DM, DM], dt.bfloat16, kind="ExternalInput")
    qkwl = nc.dram_tensor("qkwl", [DM, DM], dt.bfloat16, kind="ExternalInput")
    vw = nc.dram_tensor("vw", [DM, DG], dt.bfloat16, kind="ExternalInput")
    ow = nc.dram_tensor("ow", [DG, DM], dt.bfloat16, kind="ExternalInput")
    cs = nc.dram_tensor("cs", [P, 2, S], dt.float32, kind="ExternalInput")
    idr = nc.dram_tensor("idr", [P, P], dt.bfloat16, kind="ExternalInput")
    idrf = nc.dram_tensor("idrf", [P, P], dt.float32r, kind="ExternalInput")
    mkm = nc.dram_tensor("mkm", [P, P], dt.bfloat16, kind="ExternalInput")
    md = nc.dram_tensor("md", [P, 4, P], dt.bfloat16, kind="ExternalInput")
    on1 = nc.dram_tensor("on1", [P, DK], dt.float32r, kind="ExternalInput")
    opart = nc.dram_tensor("opart", [S, DM], dt.float32, kind="ExternalOutput")

    AluOp = mybir.AluOpType
    Act = mybir.ActivationFunctionType
    AxX = mybir.AxisListType.X

    import os
    _dma_eng = os.environ.get("K2_DMA_ENGINE", "sync")

    def sdma(out, in_):
        getattr(nc, _dma_eng).dma_start(out, in_)

    with TC(nc) as tc:
        with (
            tc.tile_pool(name="cp", bufs=1) as cp,
            tc.tile_pool(name="ps", bufs=1, space="PSUM") as ps,
        ):
            idrsb = cp.tile([P, P], dt.bfloat16, tag="idr", name="idrsb")
            idrr = cp.tile([P, P], dt.float32r, tag="idrr", name="idrr")
            mkmsb = cp.tile([P, P], dt.bfloat16, tag="mkm", name="mkmsb")
            mdsb = cp.tile([P, 4, P], dt.bfloat16, tag="md", name="mdsb")
            on1sb = cp.tile([P, DK], dt.float32r, tag="on1", name="on1sb")
            cssb = cp.tile([P, 2, S], dt.float32, tag="cssb", name="cssb")
            vaug = cp.tile([P, ST, HPG, DK + 1], dt.bfloat16, tag="vaug",
                           name="vaug")
            nc.gpsimd.memset(vaug[:, :, :, DK : DK + 1], 1.0)

            def load_consts(xp):
                # after the x chunks: cos/sin on the lightly used sync queue,
                # small constants behind the x chunks on gpsimd
                for q4 in range(4):
                    nc.sync.dma_start(cssb[:, :, q4 * 512 : (q4 + 1) * 512],
                                      cs[:, :, q4 * 512 : (q4 + 1) * 512])

                nc.gpsimd.dma_start(idrsb[:], idr[:])
                nc.gpsimd.dma_start(idrr[:], idrf[:])
                nc.gpsimd.dma_start(mkmsb[:], mkm[:])
                nc.gpsimd.dma_start(mdsb[:], md[:])
                nc.gpsimd.dma_start(on1sb[:], on1[:])
            aoT = []
            for pr in range(4):
                t_ = cp.tile([P, S], dt.bfloat16, tag=f"aoT{pr}", name=f"aoT{pr}")
                aoT.append(t_)

            packs = {}      # h -> (Kpack, KhiAug, QhiQhi, QloAug)
            pending_norm = []
            at_pool = [None]

            def emit_norm():
                while pending_norm:
                    h, J, avp, denr, dst = pending_norm.pop(0)
                    rk = ps.tile([DK, 512], dt.float32, tag="rk", bufs=1,
                                 name=f"rk{h}{J}")
                    nc.tensor.matmul(rk[:], lhsT=on1sb[DK : DK + 1, :],
                                     rhs=denr[DK : DK + 1, :],
                                     start=True, stop=True)
                    bc = at_pool[0].tile([DK, 512], dt.float32, tag="bc",
                                         bufs=1, name=f"bc{h}{J}")
                    nc.scalar.copy(bc[:], rk[:])
                    nc.vector.tensor_tensor(dst, avp[0:DK, :], bc[:],
                                            AluOp.mult)
                    if h % 2 == 1:
                        sdma(aoT[h // 2][DK:P, J * 512 : (J + 1) * 512],
                             aotmps[(h, J)][:])

            # ---------- phase A pieces ----------
            def load_inputs(xp):
                xsb, xsl = [], []
                for k in range(8):
                    eng = nc.gpsimd if k % 2 == 0 else nc.scalar
                    th = xp.tile([P, S], dt.bfloat16, tag=f"xsb{k}",
                                 name=f"xsb{k}")
                    eng.dma_start(th[:], xTh[k * P : (k + 1) * P, :])
                    xsb.append(th)
                    tl_ = xp.tile([P, S], dt.bfloat16, tag=f"xsl{k}",
                                  name=f"xsl{k}")
                    eng.dma_start(tl_[:], xTl[k * P : (k + 1) * P, :])
                    xsl.append(tl_)
                return xsb, xsl

            def v_proj(xp, xsb):
                with tc.tile_pool(name="vp", bufs=1) as vp:
                    vwsb = vp.tile([P, 8, DG], dt.bfloat16, tag="vwsb",
                                   name="vwsb")
                    nc.gpsimd.dma_start(
                        vwsb[:], vw.rearrange("(kt p) o -> p kt o", p=P))
                    for st in range(ST):
                        vps = ps.tile([P, DG], dt.float32, tag="pj", bufs=2,
                                      name=f"vps{st}")
                        for k in range(8):
                            nc.tensor.matmul(
                                vps[:],
                                lhsT=xsb[k][:, st * P : (st + 1) * P],
                                rhs=vwsb[:, k, :],
                                start=(k == 0), stop=(k == 7))
                        nc.scalar.copy(
                            vaug[:, st, :, 0:DK],
                            vps[:].rearrange("p (h d) -> p h d", d=DK))

            def proj_rope(xp, xsb, xsl, t, pr, after_weights=None):
                """Project 128 dims (t: 0=q 1=k; pr: head pair), rope, split
                hi/lo into pack tiles for heads hA=2pr, hB=2pr+1."""
                hA, hB = 2 * pr, 2 * pr + 1
                # load weight slices for this (t, pr)
                wh = xp.tile([P, 8, P], dt.bfloat16, tag="wh", bufs=2,
                             name=f"wh{t}{pr}")
                nc.sync.dma_start(
                    wh[:],
                    qkwh[:, t * DG + pr * P : t * DG + (pr + 1) * P].rearrange(
                        "(kt p) o -> p kt o", p=P))
                wl = xp.tile([P, 8, P], dt.bfloat16, tag="wl", bufs=2,
                             name=f"wl{t}{pr}")
                nc.sync.dma_start(
                    wl[:],
                    qkwl[:, t * DG + pr * P : t * DG + (pr + 1) * P].rearrange(
                        "(kt p) o -> p kt o", p=P))
                if after_weights is not None:
                    after_weights()

                # allocate pack tiles for this pair on first (t) visit
                if hA not in packs:
                    for h in (hA, hB):
                        kp = cp.tile([P, S], dt.bfloat16, tag="kpack", bufs=3,
                                     name=f"kpack{h}")
                        ka = cp.tile([DK + 2, S], dt.bfloat16, tag="khiaug",
                                     bufs=3, name=f"khiaug{h}")
                        nc.gpsimd.memset(ka[DK : DK + 2, :], 1.0)
                        qq_ = cp.tile([P, S], dt.bfloat16, tag="qhq", bufs=3,
                                      name=f"qhq{h}")
                        ql = cp.tile([DK + 2, S], dt.bfloat16, tag="qloaug",
                                     bufs=3, name=f"qloaug{h}")
                        packs[h] = (kp, ka, qq_, ql)

                stagl = xp.tile([P, S], dt.bfloat16, tag="staglo", bufs=1,
                                name=f"staglo{t}{pr}")
                stagh = (xp.tile([P, S], dt.bfloat16, tag="stagh", bufs=1,
                                 name=f"stagh{pr}") if t == 1 else None)
                kpA, kaA, qqA, qlA = packs[hA]
                kpB, kaB, qqB, qlB = packs[hB]
                terms = [(wh, xsb), (wh, xsl), (wl, xsb)]
                for cp2 in range(2):
                    pss = [ps.tile([P, 512], dt.float32, tag="pj", bufs=2,
                                   name=f"qps{t}{pr}{cp2}{i}") for i in range(2)]
                    for k in range(8):
                        for ti, (wt, xt) in enumerate(terms):
                            for i in range(2):
                                ch = 2 * cp2 + i
                                nc.tensor.matmul(
                                    pss[i][:],
                                    lhsT=wt[:, k, :],
                                    rhs=xt[k][:, ch * 512 : (ch + 1) * 512],
                                    start=(ti == 0 and k == 0),
                                    stop=(ti == 2 and k == 7))
                    for i in range(2):
                        ch = 2 * cp2 + i
                        sl = slice(ch * 512, (ch + 1) * 512)
                        qraw = xp.tile([P, 512], dt.float32, tag="qraw",
                                       bufs=2, name=f"qraw{t}{pr}{ch}")
                        nc.scalar.copy(qraw[:], pss[i][:])
                        qsw = xp.tile([P, 512], dt.float32, tag="qsw", bufs=2,
                                      name=f"qsw{t}{pr}{ch}")
                        # rotate-half swap (partition +-32 in each 64 block)
                        sdma(qsw[0:32, :], qraw[32:64, :])
                        sdma(qsw[32:64, :], qraw[0:32, :])
                        sdma(qsw[64:96, :], qraw[96:128, :])
                        sdma(qsw[96:128, :], qraw[64:96, :])
                        nc.gpsimd.tensor_tensor(qraw[:], qraw[:],
                                                cssb[:, 0, sl], AluOp.mult)
                        nc.gpsimd.tensor_tensor(qsw[:], qsw[:],
                                                cssb[:, 1, sl], AluOp.mult)
                        if t == 0:
                            # hi = bf16(rot) straight into the qq pack halves
                            nc.vector.tensor_tensor(qqA[0:64, sl],
                                                    qraw[0:64, :],
                                                    qsw[0:64, :], AluOp.add)
                            nc.vector.tensor_tensor(qqB[64:128, sl],
                                                    qraw[64:128, :],
                                                    qsw[64:128, :], AluOp.add)
                            # lo = (rot - hi): qraw -= hi, then + qsw
                            nc.vector.tensor_tensor(qraw[0:64, :],
                                                    qraw[0:64, :],
                                                    qqA[0:64, sl],
                                                    AluOp.subtract)
                            nc.vector.tensor_tensor(qraw[64:128, :],
                                                    qraw[64:128, :],
                                                    qqB[64:128, sl],
                                                    AluOp.subtract)
                            # loA direct into QloAug rows 0:64; loB staged
                            nc.vector.tensor_tensor(qlA[0:64, sl],
                                                    qraw[0:64, :],
                                                    qsw[0:64, :], AluOp.add)
                            nc.vector.tensor_tensor(stagl[64:128, sl],
                                                    qraw[64:128, :],
                                                    qsw[64:128, :], AluOp.add)
                        else:
                            # K side: full-width ops via hi staging (DVE cost
                            # is width-based, so 3 ops instead of 6)
                            nc.vector.tensor_tensor(stagh[:, sl], qraw[:],
                                                    qsw[:], AluOp.add)
                            nc.vector.tensor_tensor(qraw[:], qraw[:],
                                                    stagh[:, sl],
                                                    AluOp.subtract)
                            nc.vector.tensor_tensor(stagl[:, sl], qraw[:],
                                                    qsw[:], AluOp.add)
                # relayout DMAs
                if t == 0:
                    sdma(qqA[64:128, :], qqA[0:64, :])
                    sdma(qqB[0:64, :], qqB[64:128, :])
                    sdma(qlB[0:64, :], stagl[64:128, :])
                else:
                    sdma(kpA[0:64, :], stagh[0:64, :])
                    sdma(kpB[64:128, :], stagh[64:128, :])
                    sdma(kaA[0:DK, :], stagh[0:64, :])
                    sdma(kaB[0:DK, :], stagh[64:128, :])
                    sdma(kpA[64:128, :], stagl[0:64, :])
                    sdma(kpB[0:64, :], stagl[64:128, :])

            # ---------- phase B: attention for one head ----------
            aotmps = {}

            def pass1(at, h, J):
                """Row max of causal scores for q-block J -> QloAug aug rows."""
                kp, ka, qq_, ql = packs[h]
                Jsl = slice(J * 512, (J + 1) * 512)

                negm4 = at.tile([P, 4], dt.float32r, tag="negm4", bufs=3,
                                name=f"negm4{h}{J}")
                for qq in range(4):
                    qi = 4 * J + qq
                    kv = (qi + 1) * P
                    nch = (kv + 511) // 512
                    mparts = []
                    for c in range(nch):
                        cw = min(512, kv - c * 512)
                        sc_ = ps.tile([P, 512], dt.float32, tag="sc",
                                      bufs=2, name=f"sc{h}{qi}{c}")
                        last = c == nch - 1
                        nc.tensor.matmul(
                            sc_[:, 0:cw],
                            lhsT=qq_[:, qi * P : (qi + 1) * P],
                            rhs=kp[:, c * 512 : c * 512 + cw],
                            start=True, stop=not last)
                        if last:
                            doff = qi * P - c * 512
                            nc.tensor.matmul(
                                sc_[:, doff : doff + P],
                                lhsT=idrsb[:], rhs=mkmsb[:],
                                start=False, stop=True)
                        if nch == 1:
                            # single chunk: reduce straight into negm4 column
                            nc.vector.reduce_max(negm4[:, qq : qq + 1],
                                                 sc_[:, 0:cw], axis=AxX,
                                                 negate=True)
                        else:
                            mp = at.tile([P, 1], dt.float32r, tag="mp",
                                         bufs=8, name=f"mp{h}{qi}{c}")
                            nc.vector.reduce_max(mp[:], sc_[:, 0:cw],
                                                 axis=AxX, negate=True)
                            mparts.append(mp)
                    if nch > 1:
                        # fold the partial minima; last op lands in negm4
                        acc = mparts[0]
                        for m2_ in mparts[1:-1]:
                            nc.vector.tensor_tensor(acc[:], acc[:], m2_[:],
                                                    AluOp.min)
                        nc.vector.tensor_tensor(negm4[:, qq : qq + 1],
                                                acc[:], mparts[-1][:],
                                                AluOp.min)
                def emit_shift():
                    # transpose -max to [4,128], split to bf16 hi/lo, one
                    # linearizing DMA into each aug row of QloAug
                    ngt = ps.tile([4, P], dt.float32r, tag="ngt", bufs=1,
                                  name=f"ngt{h}{J}")
                    nc.tensor.transpose(ngt[:], negm4[:], idrr[:])
                    ngh = at.tile([4, P], dt.bfloat16, tag="ngh", bufs=3,
                                  name=f"ngh{h}{J}")
                    ngl = at.tile([4, P], dt.bfloat16, tag="ngl", bufs=3,
                                  name=f"ngl{h}{J}")
                    nc.vector.tensor_copy(ngh[:], ngt[:])
                    nc.vector.tensor_tensor(ngl[:], ngt[:], ngh[:],
                                            AluOp.subtract)
                    sdma(ql[DK : DK + 1, Jsl], ngh[:])
                    sdma(ql[DK + 1 : DK + 2, Jsl], ngl[:])
                return emit_shift

            def pass2(at, h, J, mid_cb=None):
                """Scores^T + exp + AV + denominator for q-block J."""
                kp, ka, qq_, ql = packs[h]
                pr = h // 2
                Jsl = slice(J * 512, (J + 1) * 512)
                avp = ps.tile([DK + 1, 512], dt.float32, tag="avp",
                              bufs=2, name=f"avp{h}{J}")
                nj = 4 * J + 4
                prev = []
                for j in range(nj):
                    dj = j - 4 * J
                    # columns left of a diagonal tile's triangle quarter are
                    # fully masked -- skip computing them
                    c0 = max(dj, 0) * P
                    cw = 512 - c0
                    qsl = slice(J * 512 + c0, (J + 1) * 512)
                    stp = ps.tile([P, 512], dt.float32, tag="pj", bufs=2,
                                  name=f"stp{h}{J}{j}")
                    nc.tensor.matmul(
                        stp[:, c0:512],
                        lhsT=kp[:, j * P : (j + 1) * P],
                        rhs=qq_[:, qsl],
                        start=True, stop=False)
                    nc.tensor.matmul(
                        stp[:, c0:512],
                        lhsT=ka[0 : DK + 2, j * P : (j + 1) * P],
                        rhs=ql[0 : DK + 2, qsl],
                        start=False, stop=(dj < 0),
                        skip_group_check=(dj >= 0))
                    if dj >= 0:
                        nc.tensor.matmul(
                            stp[:, c0 : c0 + P], lhsT=idrsb[:],
                            rhs=mdsb[:, dj, :],
                            start=False, stop=True, skip_group_check=True)
                    att = at.tile([P, 512], dt.bfloat16, tag="att",
                                  bufs=5, name=f"att{h}{J}{j}")
                    nc.scalar.activation(att[:, c0:512], stp[:, c0:512],
                                         Act.Exp)
                    prev.append((j, max(dj, 0) * P, att))
                    if len(prev) > 1:
                        pj_, pc0, patt = prev.pop(0)
                        nc.tensor.matmul(
                            avp[:, pc0:512], lhsT=vaug[:, pj_, h, :],
                            rhs=patt[:, pc0:512],
                            start=(pj_ == 0), stop=False,
                            skip_group_check=True)
                    if j == 1 and mid_cb is not None:
                        mid_cb()
                pj_, pc0, patt = prev.pop(0)
                nc.tensor.matmul(
                    avp[:, pc0:512], lhsT=vaug[:, pj_, h, :],
                    rhs=patt[:, pc0:512],
                    start=(pj_ == 0), stop=True, skip_group_check=True)
                denr = at.tile([DK + 1, 512], dt.float32r, tag="denr",
                               bufs=1, name=f"denr{h}{J}")
                nc.scalar.copy(denr[DK : DK + 1, :], avp[DK : DK + 1, :])
                with nc.allow_low_precision(reason="f32r recip of denom"):
                    nc.vector.reciprocal(denr[DK : DK + 1, :],
                                         denr[DK : DK + 1, :])
                if h % 2 == 0:
                    dst = aoT[pr][0:DK, Jsl]
                else:
                    dst = at.tile([DK, 512], dt.bfloat16, tag="aotmp",
                                  bufs=2, name=f"aotmp{h}{J}")
                    aotmps[(h, J)] = dst
                    dst = dst[:]
                pending_norm.append((h, J, avp, denr, dst))

            # ---------- emission ----------
            with tc.tile_pool(name="xp", bufs=1) as xp:
                with tc.tile_pool(name="at", bufs=1) as at:
                    at_pool[0] = at
                    xsb, xsl = load_inputs(xp)
                    proj_rope(xp, xsb, xsl, 1, 0,
                              after_weights=lambda: load_consts(xp))
                    proj_rope(xp, xsb, xsl, 0, 0)
                    v_proj(xp, xsb)
                    seq = [(h, J) for h in (0, 1, 2, 3, 4, 5, 7, 6)
                           for J in range(NJ)]
                    pass1(at, 0, 0)()
                    for i, (h, J) in enumerate(seq):
                        if i + 1 < len(seq):
                            pass1(at, *seq[i + 1])()
                        emit_norm()
                        pass2(at, h, J)
                        if (h, J) == (0, NJ - 1):
                            proj_rope(xp, xsb, xsl, 0, 1)
                        elif (h, J) == (1, 0):
                            proj_rope(xp, xsb, xsl, 1, 1)
                        elif (h, J) == (2, NJ - 1):
                            proj_rope(xp, xsb, xsl, 0, 2)
                        elif (h, J) == (3, 0):
                            proj_rope(xp, xsb, xsl, 1, 2)
                        elif (h, J) == (4, NJ - 1):
                            proj_rope(xp, xsb, xsl, 0, 3)
                        elif (h, J) == (5, 0):
                            proj_rope(xp, xsb, xsl, 1, 3)
                    emit_norm()

            # ---------- output projection (own pool: reuses freed space) ----
            with tc.tile_pool(name="op", bufs=1) as opp:
                owsb = []
                for pr4 in range(4):
                    t_ = opp.tile([P, DM], dt.bfloat16, tag=f"ow{pr4}",
                                  name=f"owsb{pr4}")
                    eng = nc.gpsimd if pr4 % 2 == 0 else nc.sync
                    eng.dma_start(t_[:], ow[pr4 * P : (pr4 + 1) * P, :])
                    owsb.append(t_)
                for st in range(ST):
                    ops = [ps.tile([P, 512], dt.float32, tag=tg, bufs=2,
                                   name=f"op{st}{tg}") for tg in ("pj", "sc")]
                    for pr4 in range(4):
                        for oc in range(2):
                            nc.tensor.matmul(
                                ops[oc][:],
                                lhsT=aoT[pr4][:, st * P : (st + 1) * P],
                                rhs=owsb[pr4][:, oc * 512 : (oc + 1) * 512],
                                start=(pr4 == 0), stop=(pr4 == 3))
                    osb = opp.tile([P, DM], dt.float32, tag="osb", bufs=3,
                                   name=f"osb{st}")
                    for oc in range(2):
                        nc.scalar.copy(osb[:, oc * 512 : (oc + 1) * 512],
                                       ops[oc][:])
                    sdma(opart[st * P : (st + 1) * P, :],
                                      osb[:])

    nc.compile()
    return nc


def _host_prep(x, q_w, k_w, v_w, o_w, S):
    import ml_dtypes

    perm = np.zeros(DM, dtype=np.int64)
    for h in range(H):
        for i in range(32):
            perm[h * DK + i] = h * DK + 2 * i
            perm[h * DK + 32 + i] = h * DK + 2 * i + 1
    q_wp = (q_w[perm] * 0.125).astype(np.float32)
    k_wp = k_w[perm].astype(np.float32)

    inv_freq = 1.0 / THETA ** (2.0 * np.arange(32, dtype=np.float64) / DK)
    pos = np.arange(S, dtype=np.float64)
    ang = inv_freq[:, None] * pos[None, :]
    cos = np.cos(ang).astype(np.float32)
    sin = np.sin(ang).astype(np.float32)
    cs = np.zeros((P, 2, S), dtype=np.float32)
    for blk in range(2):
        b0 = blk * DK
        cs[b0 : b0 + 32, 0] = cos
        cs[b0 + 32 : b0 + 64, 0] = cos
        cs[b0 : b0 + 32, 1] = -sin
        cs[b0 + 32 : b0 + 64, 1] = sin

    bf = ml_dtypes.bfloat16
    idr = np.eye(P, dtype=np.float32).astype(bf)
    idrf = np.eye(P, dtype=np.float32)
    r = np.arange(P)
    mkm = np.where(r[None, :] > r[:, None], np.float32(MASK_VAL),
                   np.float32(0.0)).astype(bf)
    # only each diagonal tile's own 128-wide triangle quarter is read
    md = np.zeros((P, 4, P), dtype=np.float32)
    tri = np.where(r[None, :] < r[:, None], np.float32(MASK_VAL),
                   np.float32(0.0))
    for dj in range(4):
        md[:, dj, :] = tri
    md = md.astype(bf)
    on1 = np.ones((P, DK), dtype=np.float32)

    in_maps = []
    for b in range(B):
        for g in range(NG):
            rows = slice(g * DG, (g + 1) * DG)
            xt = np.ascontiguousarray(x[b].T)
            xth = xt.astype(bf)
            qkwf = np.ascontiguousarray(
                np.concatenate([q_wp[rows].T, k_wp[rows].T], axis=1))
            qkwhh = qkwf.astype(bf)
            in_maps.append({
                "xTh": xth,
                "xTl": (xt - xth.astype(np.float32)).astype(bf),
                "qkwh": qkwhh,
                "qkwl": (qkwf - qkwhh.astype(np.float32)).astype(bf),
                "vw": np.ascontiguousarray(v_w[rows].T).astype(bf),
                "ow": np.ascontiguousarray(o_w[:, rows].T).astype(bf),
                "cs": cs,
                "idr": idr,
                "idrf": idrf,
                "mkm": mkm,
                "md": md,
                "on1": on1,
            })
    return in_maps


_NC_CACHE = {}


def kernel(x, q_w, k_w, v_w, o_w):
    import sys

    for p in ("/opt/trn_rl_repo",):
        if p not in sys.path:
            sys.path.insert(0, p)
    from concourse.bass_utils import run_bass_kernel_spmd

    x = np.asarray(x, dtype=np.float32)
    q_w = np.asarray(q_w, dtype=np.float32)
    k_w = np.asarray(k_w, dtype=np.float32)
    v_w = np.asarray(v_w, dtype=np.float32)
    o_w = np.asarray(o_w, dtype=np.float32)
    S = x.shape[1]

    if S not in _NC_CACHE:
        _NC_CACHE[S] = build_nc(S)
    nc = _NC_CACHE[S]

    in_maps = _host_prep(x, q_w, k_w, v_w, o_w, S)
    res = run_bass_kernel_spmd(nc, in_maps, core_ids=list(range(8)))

    out = np.zeros((B, S, DM), dtype=np.float32)
    for b in range(B):
        for g in range(NG):
            out[b] += res.results[b * NG + g]["opart"]
    return out


# revision 7
# speedup vs baseline: 1.0118x; 1.0114x over previous
"""Causal MHA (B=4, S=2048, D=1024, H=16) on 8 TRN2 cores — v2.

Core = (batch, head-group of 8 heads); each core computes its batch's
attention for its heads plus the partial output projection; the host
sums the two per-batch partials. ~600us/core (baseline 1032us).

Key ideas vs the v1 baseline:
  - Matmul cost on TRN2 is output-width based and independent of the
    contraction depth, so the unused 64 PE partitions carry the second
    hi/lo split term: rope'd Q/K live in SBUF as packed bf16 tiles
      Kpack   = [Khi; Klo]            QhiQhi = [Qhi; Qhi]
      KhiAug  = [Khi; 1; 1] (66 rows) QloAug = [Qlo; -mh; -ml]
    and pass-2 scores take 2 matmuls instead of 3:
      m1: Kpack x QhiQhi -> Khi.Qhi + Klo.Qhi
      m2: KhiAug x QloAug -> Khi.Qlo + shift
    (the softmax shift is a bf16 hi+lo pair since scores reach ~22k;
    pass-1 row max = QhiQhi x Kpack = Qhi.K, one matmul per tile).
  - No DRAM roundtrip: rope writes hi terms straight into pack tiles
    (Q) or a staging tile (K, full-width DVE ops); lo/dup/aug rows
    move via SBUF-SBUF DMAs on the idle SP HWDGE queue (~.6us) rather
    than Pool SWDGE (~1us of Pool engine each).
  - Diagonal kv-tiles only compute columns >= their triangle quarter
    (the rest is fully masked); masks are 128-wide quarter matmuls.
  - Emission is a software pipeline: pass-1 of (h,J+1), deferred
    normalization of (h,J-1) (rank-1 1/den broadcast via matmul), then
    pass-2+AV of (h,J); projections+rope of the next head pair are
    interleaved at fixed points so pack rings stay at 3 buffers while
    the PE never starves; per-chunk x loads feed the first projection
    as they arrive.
"""

import numpy as np

P = 128
B, S_FULL, DM = 4, 2048, 1024
H, DK = 16, 64
NG = 2
HPG = 8
DG = 512
THETA = 10000.0
MASK_VAL = -1e9


def build_nc(S):
    import concourse.bacc as bacc
    import concourse.mybir as mybir
    import concourse.tile as tile

    dt = mybir.dt
    ST = S // P
    NJ = ST // 4
    SC = S // 512

    TC = tile.TileContext
    nc = bacc.Bacc(None, target_bir_lowering=False)

    xTh = nc.dram_tensor("xTh", [DM, S], dt.bfloat16, kind="ExternalInput")
    xTl = nc.dram_tensor("xTl", [DM, S], dt.bfloat16, kind="ExternalInput")
    qkwh = nc.dram_tensor("qkwh", [DM, DM], dt.bfloat16, kind="ExternalInput")
    qkwl = nc.dram_tensor("qkwl", [DM, DM], dt.bfloat16, kind="ExternalInput")
    vw = nc.dram_tensor("vw", [DM, DG], dt.bfloat16, kind="ExternalInput")
    ow = nc.dram_tensor("ow", [DG, DM], dt.bfloat16, kind="ExternalInput")
    cs = nc.dram_tensor("cs", [P, 2, S], dt.float32, kind="ExternalInput")
    idr = nc.dram_tensor("idr", [P, P], dt.bfloat16, kind="ExternalInput")
    idrf = nc.dram_tensor("idrf", [P, P], dt.float32r, kind="ExternalInput")
    mkm = nc.dram_tensor("mkm", [P, P], dt.bfloat16, kind="ExternalInput")
    md = nc.dram_tensor("md", [P, 4, P], dt.bfloat16, kind="ExternalInput")
    on1 = nc.dram_tensor("on1", [P, DK], dt.float32r, kind="ExternalInput")
    opart = nc.dram_tensor("opart", [S, DM], dt.float32, kind="ExternalOutput")

    AluOp = mybir.AluOpType
    Act = mybir.ActivationFunctionType
    AxX = mybir.AxisListType.X

    import os
    _dma_eng = os.environ.get("K2_DMA_ENGINE", "sync")

    def sdma(out, in_):
        getattr(nc, _dma_eng).dma_start(out, in_)

    with TC(nc) as tc:
        with (
            tc.tile_pool(name="cp", bufs=1) as cp,
            tc.tile_pool(name="ps", bufs=1, space="PSUM") as ps,
        ):
            idrsb = cp.tile([P, P], dt.bfloat16, tag="idr", name="idrsb")
            idrr = cp.tile([P, P], dt.float32r, tag="idrr", name="idrr")
            mkmsb = cp.tile([P, P], dt.bfloat16, tag="mkm", name="mkmsb")
            mdsb = cp.tile([P, 4, P], dt.bfloat16, tag="md", name="mdsb")
            on1sb = cp.tile([P, DK], dt.float32r, tag="on1", name="on1sb")
            cssb = cp.tile([P, 2, S], dt.float32, tag="cssb", name="cssb")
            vaug = cp.tile([P, ST, HPG, DK + 1], dt.bfloat16, tag="vaug",
                           name="vaug")
            nc.gpsimd.memset(vaug[:, :, :, DK : DK + 1], 1.0)

            def load_consts(xp):
                # after the x chunks: cos/sin on the lightly used sync queue,
                # small constants behind the x chunks on gpsimd
                for q4 in range(4):
                    nc.sync.dma_start(cssb[:, :, q4 * 512 : (q4 + 1) * 512],
                                      cs[:, :, q4 * 512 : (q4 + 1) * 512])

                nc.gpsimd.dma_start(idrsb[:], idr[:])
                nc.gpsimd.dma_start(idrr[:], idrf[:])
                nc.gpsimd.dma_start(mkmsb[:], mkm[:])
                nc.gpsimd.dma_start(mdsb[:], md[:])
                nc.gpsimd.dma_start(on1sb[:], on1[:])
            aoT = []
            for pr in range(4):
                t_ = cp.tile([P, S], dt.bfloat16, tag=f"aoT{pr}", name=f"aoT{pr}")
                aoT.append(t_)

            packs = {}      # h -> (Kpack, KhiAug, QhiQhi, QloAug)
            pending_norm = []
            at_pool = [None]

            def emit_norm():
                while pending_norm:
                    h, J, avp, denr, dst = pending_norm.pop(0)
                    rk = ps.tile([DK, 512], dt.float32, tag="rk", bufs=1,
                                 name=f"rk{h}{J}")
                    nc.tensor.matmul(rk[:], lhsT=on1sb[DK : DK + 1, :],
                                     rhs=denr[DK : DK + 1, :],
                                     start=True, stop=True)
                    bc = at_pool[0].tile([DK, 512], dt.float32, tag="bc",
                                         bufs=1, name=f"bc{h}{J}")
                    nc.scalar.copy(bc[:], rk[:])
                    nc.vector.tensor_tensor(dst, avp[0:DK, :], bc[:],
                                            AluOp.mult)
                    if h % 2 == 1:
                        sdma(aoT[h // 2][DK:P, J * 512 : (J + 1) * 512],
                             aotmps[(h, J)][:])

            # ---------- phase A pieces ----------
            def load_inputs(xp):
                xsb, xsl = [], []
                for k in range(8):
                    eng = nc.gpsimd if k % 2 == 0 else nc.scalar
                    th = xp.tile([P, S], dt.bfloat16, tag=f"xsb{k}",
                                 name=f"xsb{k}")
                    eng.dma_start(th[:], xTh[k * P : (k + 1) * P, :])
                    xsb.append(th)
                    tl_ = xp.tile([P, S], dt.bfloat16, tag=f"xsl{k}",
                                  name=f"xsl{k}")
                    eng.dma_start(tl_[:], xTl[k * P : (k + 1) * P, :])
                    xsl.append(tl_)
                return xsb, xsl

            def v_proj(xp, xsb):
                with tc.tile_pool(name="vp", bufs=1) as vp:
                    vwsb = vp.tile([P, 8, DG], dt.bfloat16, tag="vwsb",
                                   name="vwsb")
                    nc.gpsimd.dma_start(
                        vwsb[:], vw.rearrange("(kt p) o -> p kt o", p=P))
                    for st in range(ST):
                        vps = ps.tile([P, DG], dt.float32, tag="pj", bufs=2,
                                      name=f"vps{st}")
                        for k in range(8):
                            nc.tensor.matmul(
                                vps[:],
                                lhsT=xsb[k][:, st * P : (st + 1) * P],
                                rhs=vwsb[:, k, :],
                                start=(k == 0), stop=(k == 7))
                        nc.scalar.copy(
                            vaug[:, st, :, 0:DK],
                            vps[:].rearrange("p (h d) -> p h d", d=DK))

            def proj_rope(xp, xsb, xsl, t, pr, after_weights=None):
                """Project 128 dims (t: 0=q 1=k; pr: head pair), rope, split
                hi/lo into pack tiles for heads hA=2pr, hB=2pr+1."""
                hA, hB = 2 * pr, 2 * pr + 1
                # load weight slices for this (t, pr)
                wh = xp.tile([P, 8, P], dt.bfloat16, tag="wh", bufs=2,
                             name=f"wh{t}{pr}")
                nc.sync.dma_start(
                    wh[:],
                    qkwh[:, t * DG + pr * P : t * DG + (pr + 1) * P].rearrange(
                        "(kt p) o -> p kt o", p=P))
                wl = xp.tile([P, 8, P], dt.bfloat16, tag="wl", bufs=2,
                             name=f"wl{t}{pr}")
                nc.sync.dma_start(
                    wl[:],
                    qkwl[:, t * DG + pr * P : t * DG + (pr + 1) * P].rearrange(
                        "(kt p) o -> p kt o", p=P))
                if after_weights is not None:
                    after_weights()

                # allocate pack tiles for this pair on first (t) visit
                if hA not in packs:
                    for h in (hA, hB):
                        kp = cp.tile([P, S], dt.bfloat16, tag="kpack", bufs=3,
                                     name=f"kpack{h}")
                        ka = cp.tile([DK + 2, S], dt.bfloat16, tag="khiaug",
                                     bufs=3, name=f"khiaug{h}")
                        nc.gpsimd.memset(ka[DK : DK + 2, :], 1.0)
                        qq_ = cp.tile([P, S], dt.bfloat16, tag="qhq", bufs=3,
                                      name=f"qhq{h}")
                        ql = cp.tile([DK + 2, S], dt.bfloat16, tag="qloaug",
                                     bufs=3, name=f"qloaug{h}")
                        packs[h] = (kp, ka, qq_, ql)

                stagl = xp.tile([P, S], dt.bfloat16, tag="staglo", bufs=1,
                                name=f"staglo{t}{pr}")
                stagh = (xp.tile([P, S], dt.bfloat16, tag="stagh", bufs=1,
                                 name=f"stagh{pr}") if t == 1 else None)
                kpA, kaA, qqA, qlA = packs[hA]
                kpB, kaB, qqB, qlB = packs[hB]
                terms = [(wh, xsb), (wh, xsl), (wl, xsb)]
                for cp2 in range(2):
                    pss = [ps.tile([P, 512], dt.float32, tag="pj", bufs=2,
                                   name=f"qps{t}{pr}{cp2}{i}") for i in range(2)]
                    for k in range(8):
                        for ti, (wt, xt) in enumerate(terms):
                            for i in range(2):
                                ch = 2 * cp2 + i
                                nc.tensor.matmul(
                                    pss[i][:],
                                    lhsT=wt[:, k, :],
                                    rhs=xt[k][:, ch * 512 : (ch + 1) * 512],
                                    start=(ti == 0 and k == 0),
                                    stop=(ti == 2 and k == 7))
                    for i in range(2):
                        ch = 2 * cp2 + i
                        sl = slice(ch * 512, (ch + 1) * 512)
                        qraw = xp.tile([P, 512], dt.float32, tag="qraw",
                                       bufs=2, name=f"qraw{t}{pr}{ch}")
                        nc.scalar.copy(qraw[:], pss[i][:])
                        qsw = xp.tile([P, 512], dt.float32, tag="qsw", bufs=2,
                                      name=f"qsw{t}{pr}{ch}")
                        # rotate-half swap (partition +-32 in each 64 block)
                        sdma(qsw[0:32, :], qraw[32:64, :])
                        sdma(qsw[32:64, :], qraw[0:32, :])
                        sdma(qsw[64:96, :], qraw[96:128, :])
                        sdma(qsw[96:128, :], qraw[64:96, :])
                        nc.gpsimd.tensor_tensor(qraw[:], qraw[:],
                                                cssb[:, 0, sl], AluOp.mult)
                        nc.gpsimd.tensor_tensor(qsw[:], qsw[:],
                                                cssb[:, 1, sl], AluOp.mult)
                        if t == 0:
                            # hi = bf16(rot) straight into the qq pack halves
                            nc.vector.tensor_tensor(qqA[0:64, sl],
                                                    qraw[0:64, :],
                                                    qsw[0:64, :], AluOp.add)
                            nc.vector.tensor_tensor(qqB[64:128, sl],
                                                    qraw[64:128, :],
                                                    qsw[64:128, :], AluOp.add)
                            # lo = (rot - hi): qraw -= hi, then + qsw
                            nc.vector.tensor_tensor(qraw[0:64, :],
                                                    qraw[0:64, :],
                                                    qqA[0:64, sl],
                                                    AluOp.subtract)
                            nc.vector.tensor_tensor(qraw[64:128, :],
                                                    qraw[64:128, :],
                                                    qqB[64:128, sl],
                                                    AluOp.subtract)
                            # loA direct into QloAug rows 0:64; loB staged
                            nc.vector.tensor_tensor(qlA[0:64, sl],
                                                    qraw[0:64, :],
                                                    qsw[0:64, :], AluOp.add)
                            nc.vector.tensor_tensor(stagl[64:128, sl],
                                                    qraw[64:128, :],
                                                    qsw[64:128, :], AluOp.add)
                        else:
                            # K side: full-width ops via hi staging (DVE cost
                            # is width-based, so 3 ops instead of 6)
                            nc.vector.tensor_tensor(stagh[:, sl], qraw[:],
                                                    qsw[:], AluOp.add)
                            nc.vector.tensor_tensor(qraw[:], qraw[:],
                                                    stagh[:, sl],
                                                    AluOp.subtract)
                            nc.vector.tensor_tensor(stagl[:, sl], qraw[:],
                                                    qsw[:], AluOp.add)
                # relayout DMAs
                if t == 0:
                    sdma(qqA[64:128, :], qqA[0:64, :])
                    sdma(qqB[0:64, :], qqB[64:128, :])
                    sdma(qlB[0:64, :], stagl[64:128, :])
                else:
                    sdma(kpA[0:64, :], stagh[0:64, :])
                    sdma(kpB[64:128, :], stagh[64:128, :])
                    sdma(kaA[0:DK, :], stagh[0:64, :])
                    sdma(kaB[0:DK, :], stagh[64:128, :])
                    sdma(kpA[64:128, :], stagl[0:64, :])
                    sdma(kpB[0:64, :], stagl[64:128, :])

            # ---------- phase B: attention for one head ----------
            aotmps = {}

            def pass1(at, h, J):
                """Row max of causal scores for q-block J -> QloAug aug rows."""
                kp, ka, qq_, ql = packs[h]
                Jsl = slice(J * 512, (J + 1) * 512)

                negm4 = at.tile([P, 4], dt.float32r, tag="negm4", bufs=3,
                                name=f"negm4{h}{J}")
                for qq in range(4):
                    qi = 4 * J + qq
                    kv = (qi + 1) * P
                    nch = (kv + 511) // 512
                    mparts = []
                    for c in range(nch):
                        cw = min(512, kv - c * 512)
                        sc_ = ps.tile([P, 512], dt.float32, tag="sc",
                                      bufs=2, name=f"sc{h}{qi}{c}")
                        last = c == nch - 1
                        nc.tensor.matmul(
                            sc_[:, 0:cw],
                            lhsT=qq_[:, qi * P : (qi + 1) * P],
                            rhs=kp[:, c * 512 : c * 512 + cw],
                            start=True, stop=not last)
                        if last:
                            doff = qi * P - c * 512
                            nc.tensor.matmul(
                                sc_[:, doff : doff + P],
                                lhsT=idrsb[:], rhs=mkmsb[:],
                                start=False, stop=True)
                        if nch == 1:
                            # single chunk: reduce straight into negm4 column
                            nc.vector.reduce_max(negm4[:, qq : qq + 1],
                                                 sc_[:, 0:cw], axis=AxX,
                                                 negate=True)
                        else:
                            mp = at.tile([P, 1], dt.float32r, tag="mp",
                                         bufs=8, name=f"mp{h}{qi}{c}")
                            nc.vector.reduce_max(mp[:], sc_[:, 0:cw],
                                                 axis=AxX, negate=True)
                            mparts.append(mp)
                    if nch > 1:
                        # fold the partial minima; last op lands in negm4
                        acc = mparts[0]
                        for m2_ in mparts[1:-1]:
                            nc.vector.tensor_tensor(acc[:], acc[:], m2_[:],
                                                    AluOp.min)
                        nc.vector.tensor_tensor(negm4[:, qq : qq + 1],
                                                acc[:], mparts[-1][:],
                                                AluOp.min)
                def emit_shift():
                    # transpose -max to [4,128], split to bf16 hi/lo, one
                    # linearizing DMA into each aug row of QloAug
                    ngt = ps.tile([4, P], dt.float32r, tag="ngt", bufs=1,
                                  name=f"ngt{h}{J}")
                    nc.tensor.transpose(ngt[:], negm4[:], idrr[:])
                    ngh = at.tile([4, P], dt.bfloat16, tag="ngh", bufs=3,
                                  name=f"ngh{h}{J}")
                    ngl = at.tile([4, P], dt.bfloat16, tag="ngl", bufs=3,
                                  name=f"ngl{h}{J}")
                    nc.vector.tensor_copy(ngh[:], ngt[:])
                    nc.vector.tensor_tensor(ngl[:], ngt[:], ngh[:],
                                            AluOp.subtract)
                    sdma(ql[DK : DK + 1, Jsl], ngh[:])
                    sdma(ql[DK + 1 : DK + 2, Jsl], ngl[:])
                return emit_shift

            def pass2(at, h, J, mid_cb=None):
                """Scores^T + exp + AV + denominator for q-block J."""
                kp, ka, qq_, ql = packs[h]
                pr = h // 2
                Jsl = slice(J * 512, (J + 1) * 512)
                avp = ps.tile([DK + 1, 512], dt.float32, tag="avp",
                              bufs=2, name=f"avp{h}{J}")
                nj = 4 * J + 4
                prev = []
                for j in range(nj):
                    dj = j - 4 * J
                    # columns left of a diagonal tile's triangle quarter are
                    # fully masked -- skip computing them
                    c0 = max(dj, 0) * P
                    cw = 512 - c0
                    qsl = slice(J * 512 + c0, (J + 1) * 512)
                    stp = ps.tile([P, 512], dt.float32, tag="pj", bufs=2,
                                  name=f"stp{h}{J}{j}")
                    nc.tensor.matmul(
                        stp[:, c0:512],
                        lhsT=kp[:, j * P : (j + 1) * P],
                        rhs=qq_[:, qsl],
                        start=True, stop=False)
                    nc.tensor.matmul(
                        stp[:, c0:512],
                        lhsT=ka[0 : DK + 2, j * P : (j + 1) * P],
                        rhs=ql[0 : DK + 2, qsl],
                        start=False, stop=(dj < 0),
                        skip_group_check=(dj >= 0))
                    if dj >= 0:
                        nc.tensor.matmul(
                            stp[:, c0 : c0 + P], lhsT=idrsb[:],
                            rhs=mdsb[:, dj, :],
                            start=False, stop=True, skip_group_check=True)
                    att = at.tile([P, 512], dt.bfloat16, tag="att",
                                  bufs=5, name=f"att{h}{J}{j}")
                    nc.scalar.activation(att[:, c0:512], stp[:, c0:512],
                                         Act.Exp)
                    prev.append((j, max(dj, 0) * P, att))
                    if len(prev) > 1:
                        pj_, pc0, patt = prev.pop(0)
                        nc.tensor.matmul(
                            avp[:, pc0:512], lhsT=vaug[:, pj_, h, :],
                            rhs=patt[:, pc0:512],
                            start=(pj_ == 0), stop=False,
                            skip_group_check=True)
                    if j == 1 and mid_cb is not None:
                        mid_cb()
                pj_, pc0, patt = prev.pop(0)
                nc.tensor.matmul(
                    avp[:, pc0:512], lhsT=vaug[:, pj_, h, :],
                    rhs=patt[:, pc0:512],
                    start=(pj_ == 0), stop=True, skip_group_check=True)
                denr = at.tile([DK + 1, 512], dt.float32r, tag="denr",
                               bufs=1, name=f"denr{h}{J}")
                nc.scalar.copy(denr[DK : DK + 1, :], avp[DK : DK + 1, :])
                with nc.allow_low_precision(reason="f32r recip of denom"):
                    nc.vector.reciprocal(denr[DK : DK + 1, :],
                                         denr[DK : DK + 1, :])
                if h % 2 == 0:
                    dst = aoT[pr][0:DK, Jsl]
                else:
                    dst = at.tile([DK, 512], dt.bfloat16, tag="aotmp",
                                  bufs=2, name=f"aotmp{h}{J}")
                    aotmps[(h, J)] = dst
                    dst = dst[:]
                pending_norm.append((h, J, avp, denr, dst))

            # ---------- emission ----------
            with tc.tile_pool(name="xp", bufs=1) as xp:
                with tc.tile_pool(name="at", bufs=1) as at:
                    at_pool[0] = at
                    xsb, xsl = load_inputs(xp)
                    proj_rope(xp, xsb, xsl, 1, 0,
                              after_weights=lambda: load_consts(xp))
                    proj_rope(xp, xsb, xsl, 0, 0)
                    v_proj(xp, xsb)
                    seq = [(h, J) for h in (0, 1, 2, 3, 4, 5, 7, 6)
                           for J in range(NJ)]
                    pass1(at, *seq[0])()
                    pass1(at, *seq[1])()
                    for i, (h, J) in enumerate(seq):
                        if i + 2 < len(seq):
                            pass1(at, *seq[i + 2])()
                        emit_norm()
                        pass2(at, h, J)
                        if (h, J) == (0, NJ - 1):
                            proj_rope(xp, xsb, xsl, 0, 1)
                        elif (h, J) == (1, 0):
                            proj_rope(xp, xsb, xsl, 1, 1)
                        elif (h, J) == (2, NJ - 1):
                            proj_rope(xp, xsb, xsl, 0, 2)
                        elif (h, J) == (3, 0):
                            proj_rope(xp, xsb, xsl, 1, 2)
                        elif (h, J) == (4, NJ - 1):
                            proj_rope(xp, xsb, xsl, 0, 3)
                        elif (h, J) == (5, 0):
                            proj_rope(xp, xsb, xsl, 1, 3)
                    emit_norm()

            # ---------- output projection (own pool: reuses freed space) ----
            with tc.tile_pool(name="op", bufs=1) as opp:
                owsb = []
                for pr4 in range(4):
                    t_ = opp.tile([P, DM], dt.bfloat16, tag=f"ow{pr4}",
                                  name=f"owsb{pr4}")
                    eng = nc.gpsimd if pr4 % 2 == 0 else nc.sync
                    eng.dma_start(t_[:], ow[pr4 * P : (pr4 + 1) * P, :])
                    owsb.append(t_)
                for st in range(ST):
                    ops = [ps.tile([P, 512], dt.float32, tag=tg, bufs=2,
                                   name=f"op{st}{tg}") for tg in ("pj", "sc")]
                    for pr4 in range(4):
                        for oc in range(2):
                            nc.tensor.matmul(
                                ops[oc][:],
                                lhsT=aoT[pr4][:, st * P : (st + 1) * P],
                                rhs=owsb[pr4][:, oc * 512 : (oc + 1) * 512],
                                start=(pr4 == 0), stop=(pr4 == 3))
                    osb = opp.tile([P, DM], dt.float32, tag="osb", bufs=3,
                                   name=f"osb{st}")
                    for oc in range(2):
                        nc.scalar.copy(osb[:, oc * 512 : (oc + 1) * 512],
                                       ops[oc][:])
                    sdma(opart[st * P : (st + 1) * P, :],
                                      osb[:])

    nc.compile()
    return nc


def _host_prep(x, q_w, k_w, v_w, o_w, S):
    import ml_dtypes

    perm = np.zeros(DM, dtype=np.int64)
    for h in range(H):
        for i in range(32):
            perm[h * DK + i] = h * DK + 2 * i
            perm[h * DK + 32 + i] = h * DK + 2 * i + 1
    q_wp = (q_w[perm] * 0.125).astype(np.float32)
    k_wp = k_w[perm].astype(np.float32)

    inv_freq = 1.0 / THETA ** (2.0 * np.arange(32, dtype=np.float64) / DK)
    pos = np.arange(S, dtype=np.float64)
    ang = inv_freq[:, None] * pos[None, :]
    cos = np.cos(ang).astype(np.float32)
    sin = np.sin(ang).astype(np.float32)
    cs = np.zeros((P, 2, S), dtype=np.float32)
    for blk in range(2):
        b0 = blk * DK
        cs[b0 : b0 + 32, 0] = cos
        cs[b0 + 32 : b0 + 64, 0] = cos
        cs[b0 : b0 + 32, 1] = -sin
        cs[b0 + 32 : b0 + 64, 1] = sin

    bf = ml_dtypes.bfloat16
    idr = np.eye(P, dtype=np.float32).astype(bf)
    idrf = np.eye(P, dtype=np.float32)
    r = np.arange(P)
    mkm = np.where(r[None, :] > r[:, None], np.float32(MASK_VAL),
                   np.float32(0.0)).astype(bf)
    # only each diagonal tile's own 128-wide triangle quarter is read
    md = np.zeros((P, 4, P), dtype=np.float32)
    tri = np.where(r[None, :] < r[:, None], np.float32(MASK_VAL),
                   np.float32(0.0))
    for dj in range(4):
        md[:, dj, :] = tri
    md = md.astype(bf)
    on1 = np.ones((P, DK), dtype=np.float32)

    in_maps = []
    for b in range(B):
        for g in range(NG):
            rows = slice(g * DG, (g + 1) * DG)
            xt = np.ascontiguousarray(x[b].T)
            xth = xt.astype(bf)
            qkwf = np.ascontiguousarray(
                np.concatenate([q_wp[rows].T, k_wp[rows].T], axis=1))
            qkwhh = qkwf.astype(bf)
            in_maps.append({
                "xTh": xth,
                "xTl": (xt - xth.astype(np.float32)).astype(bf),
                "qkwh": qkwhh,
                "qkwl": (qkwf - qkwhh.astype(np.float32)).astype(bf),
                "vw": np.ascontiguousarray(v_w[rows].T).astype(bf),
                "ow": np.ascontiguousarray(o_w[:, rows].T).astype(bf),
                "cs": cs,
                "idr": idr,
                "idrf": idrf,
                "mkm": mkm,
                "md": md,
                "on1": on1,
            })
    return in_maps


_NC_CACHE = {}


def kernel(x, q_w, k_w, v_w, o_w):
    import sys

    for p in ("/opt/trn_rl_repo",):
        if p not in sys.path:
            sys.path.insert(0, p)
    from concourse.bass_utils import run_bass_kernel_spmd

    x = np.asarray(x, dtype=np.float32)
    q_w = np.asarray(q_w, dtype=np.float32)
    k_w = np.asarray(k_w, dtype=np.float32)
    v_w = np.asarray(v_w, dtype=np.float32)
    o_w = np.asarray(o_w, dtype=np.float32)
    S = x.shape[1]

    if S not in _NC_CACHE:
        _NC_CACHE[S] = build_nc(S)
    nc = _NC_CACHE[S]

    in_maps = _host_prep(x, q_w, k_w, v_w, o_w, S)
    res = run_bass_kernel_spmd(nc, in_maps, core_ids=list(range(8)))

    out = np.zeros((B, S, DM), dtype=np.float32)
    for b in range(B):
        for g in range(NG):
            out[b] += res.results[b * NG + g]["opart"]
    return out
